# revision 1
# baseline (speedup 1.0000x reference)
"""Trainium2 Bass kernel for nn_Conv_layer_14276471292174 (GNN message passing).

out[b,v] = relu( sum_k feats[b,v,k,:] @ Wk^T + bias ),  Wk[o,c] = W[o, c*17+k],
feats slot 0 = self feature, slots 1..16 = gathered neighbor features.

Strategy: data-parallel over batch (1 batch element per NeuronCore, 8 cores).
Per core, fully fp32-exact:
  - neighbor gather via SWDGE indirect DMA (the only HW-validated dynamic-DMA
    primitive in this environment): one call gathers 128 rows of
    feature_map[b] (256B each) -- one row per SBUF partition, indices taken
    from an int32 [128,1] SBUF column.  17 slots x 32 m-chunks = 544 calls.
  - each gathered [128v, 64c] tile is PE-transposed to [64c, 128v] (psum),
    copied to SBUF, and used as the f32 matmul moving operand against the
    stationary Wk^T [64c, 64o]; 17 slots accumulate into a PSUM bank slice.
  - epilogue per 512-wide vertex tile: ScalarE relu(x+bias), PE transpose back
    to vertex-major, DVE copy, contiguous DMA to DRAM.

Vertex u is processed at gather position (p=u//32, m=u%32) so output tiles land
in natural vertex order with 1KB-contiguous per-partition output DMAs.
"""

import numpy as np

import concourse.bacc as bacc
import concourse.bass as bass
import concourse.mybir as mybir
import concourse.tile as tile
from concourse.bass_utils import run_bass_kernel_spmd
from concourse.masks import make_identity

BS, V, N, C, O = 8, 4096, 16, 64, 64
K = N + 1          # 17 slots: self + 16 neighbors
P = 128            # partitions
T = V // P         # 32 m-chunks ("(p m)" vertex layout)
VTS = 512          # psum accumulator width (vertex tile)
VT = V // VTS      # 8 vertex tiles
QM = VTS // P      # 4 m-chunks per vertex tile

F32DT = mybir.dt.float32
I32DT = mybir.dt.int32

_NC = None
TRACE = False
LAST_RESULT = None


def _build_nc():
    nc = bacc.Bacc()

    fm = nc.dram_tensor("fm", [V, C], F32DT, kind="ExternalInput")
    idxd = nc.dram_tensor("idxd", [P, K, T], I32DT, kind="ExternalInput")
    wtd = nc.dram_tensor("wtd", [C, K, O], F32DT, kind="ExternalInput")
    biasd = nc.dram_tensor("biasd", [O, 1], F32DT, kind="ExternalInput")
    outd = nc.dram_tensor("outd", [V, O], F32DT, kind="ExternalOutput")

    with tile.TileContext(nc) as tc:
        with (
            tc.tile_pool(name="singles", bufs=1) as singles,
            tc.tile_pool(name="gpool", bufs=8) as gpool,
            tc.tile_pool(name="rpool", bufs=4) as rpool,
            tc.tile_pool(name="psum", bufs=4, space="PSUM") as psum,
            tc.tile_pool(name="opool", bufs=3) as opool,
        ):
            IDX = singles.tile([P, K, T], I32DT)
            WT = singles.tile([C, K, O], F32DT)
            BIAS = singles.tile([O, 1], F32DT)
            ident = singles.tile([P, P], F32DT)
            ident64 = singles.tile([O, O], F32DT)

            nc.sync.dma_start(out=IDX, in_=idxd.ap())
            nc.sync.dma_start(out=WT, in_=wtd.ap())
            nc.sync.dma_start(out=BIAS, in_=biasd.ap())
            make_identity(nc, ident[:, :])
            make_identity(nc, ident64[:, :])

            out_r = outd.ap().rearrange("(p m) c -> p m c", p=P)

            for vt in range(VT):
                acc = psum.tile([O, VTS], F32DT, tag="acc", name=f"acc{vt}")
                for q in range(QM):
                    m = vt * QM + q
                    for k in range(K):
                        g = gpool.tile([P, C], F32DT, tag="g", name=f"g{m}_{k}")
                        nc.gpsimd.indirect_dma_start(
                            out=g[:, :],
                            out_offset=None,
                            in_=fm.ap(),
                            in_offset=bass.IndirectOffsetOnAxis(
                                ap=IDX[:, k, m : m + 1], axis=0
                            ),
                        )
                        gt = psum.tile([P, P], F32DT, tag="tp", name=f"t{m}_{k}")
                        nc.tensor.transpose(
                            out=gt[:C, :], in_=g[:, :], identity=ident[:, :]
                        )
                        r = rpool.tile([C, P], F32DT, tag="r", name=f"r{m}_{k}")
                        nc.vector.tensor_copy(out=r[:, :], in_=gt[:C, :])
                        nc.tensor.matmul(
                            acc[:, q * P : (q + 1) * P],
                            WT[:, k, :],
                            r[:, :],
                            start=(k == 0),
                            stop=(k == K - 1),
                        )

                # epilogue: bias+relu, transpose back, store
                av = opool.tile([O, VTS], F32DT, tag="av")
                nc.scalar.activation(
                    out=av[:, :],
                    in_=acc[:, :],
                    func=mybir.ActivationFunctionType.Relu,
                    bias=BIAS[:, 0:1],
                )
                tp = psum.tile([P, QM, O], F32DT, tag="tp", name=f"tpo{vt}")
                for j in range(QM):
                    nc.tensor.transpose(
                        out=tp[:, j, :],
                        in_=av[:, j * P : (j + 1) * P],
                        identity=ident64[:, :],
                    )
                ov = opool.tile([P, QM, O], F32DT, tag="ov")
                nc.vector.tensor_copy(out=ov[:, :, :], in_=tp[:, :, :])
                nc.sync.dma_start(
                    out=out_r[:, vt * QM : (vt + 1) * QM, :], in_=ov[:, :, :]
                )

    nc.compile()
    return nc


def _get_nc():
    global _NC
    if _NC is None:
        _NC = _build_nc()
    return _NC


def host_pack(inputs):
    """Shard inputs per core; lay out index/weight metadata."""
    nidx = np.asarray(inputs["neighbor_index"]).astype(np.int64)  # (BS, V, N)
    fmap = np.ascontiguousarray(np.asarray(inputs["feature_map"], dtype=np.float32))
    W = np.asarray(inputs["weights"], dtype=np.float32)  # (O, C*K)
    bias = np.asarray(inputs["bias"], dtype=np.float32)

    # W[o, c*K+k] -> wt[c, k, o]
    wt = np.ascontiguousarray(W.reshape(O, C, K).transpose(1, 2, 0))
    bias2 = bias.reshape(O, 1).copy()

    # vertex u handled at (p=u//T? no: p=u//32? we use p = u // T ... careful:
    # position (p, m) <-> vertex u = p*T + m
    p_arr = np.arange(P)
    in_maps = []
    for b in range(BS):
        # idx[p, k, m] = source row for slot k of vertex u = p*T + m
        u = (p_arr[:, None] * T + np.arange(T)[None, :])  # (P, T)
        idx_core = np.empty((P, K, T), np.int32)
        idx_core[:, 0, :] = u
        nb = nidx[b]  # (V, N)
        idx_core[:, 1:, :] = nb[u].transpose(0, 2, 1)  # (P, T, N) -> (P, N, T)
        in_maps.append(
            {
                "fm": fmap[b],
                "idxd": idx_core,
                "wtd": wt,
                "biasd": bias2,
            }
        )
    return in_maps


def kernel(**inputs):
    global LAST_RESULT
    in_maps = host_pack(inputs)
    nc = _get_nc()
    res = run_bass_kernel_spmd(nc, in_maps, list(range(BS)), trace=TRACE)
    LAST_RESULT = res
    out = np.stack([res.results[i]["outd"] for i in range(BS)])  # (BS, V, O)
    return out

